# revision 8
# baseline (speedup 1.0000x reference)
"""Trainium2 Bass kernel for a seq2seq CandlestickLSTM.

Model (per reference): 2-layer LSTM encoder over S=64 steps, then a
2-layer LSTM decoder run autoregressively for T=32 steps with an MLP
head (Linear(H,H/2) -> ReLU -> Linear(H/2,OUT) -> Sigmoid) whose output
feeds back as the next decoder input.

Sharding: pure data parallel over 8 NeuronCores -- batch 4096 -> 512
rows per core; all weights replicated. No collectives needed.

On-core layout: feature-major ("transposed"): activations h, c live as
[128 partitions, HT, Bc] where hidden tile k occupies page k. Matmuls
compute z.T = W @ input.T via out = lhsT.T @ rhs with host-pre-packed
weights.

Precision: the large K=256 hidden-state projections run in fp8e4m3 with
MatmulPerfMode.DoubleRow (2 fp8 K-rows per PE cycle, K=256 in a single
matmul). Weights and h are pre-scaled by 32 (products by 1024) to keep
fp8 values in the normal range; the 1/1024 unwind rides the activation
instruction's free scale operand. The tiny K=4 input projections and
the MLP second layer stay bf16 (weights pre-scaled by 1024 where they
accumulate into scaled PSUM). PSUM accumulation and pre-activation z
are fp32; gates and c are bf16. Validated end-to-end rel err ~1e-3 vs
the fp32 reference (gate 2e-2).
"""

import numpy as np
import ml_dtypes
from contextlib import ExitStack

import concourse.bass as bass
import concourse.tile as tile
from concourse import bacc, mybir
from concourse.bass_utils import run_bass_kernel_spmd

NCORES = 8
B, S, IN, H, OUT = 4096, 64, 4, 256, 4
BC = B // NCORES          # 512 batch rows per core
HT = H // 128             # 2 hidden 128-tiles
GT = 4 * H // 128         # 8 gate M-tiles
HH = H // 2               # 128 (MLP hidden)
F32 = mybir.dt.float32
BF16 = mybir.dt.bfloat16
F8 = mybir.dt.float8e4
AF = mybir.ActivationFunctionType
ALU = mybir.AluOpType
DR = mybir.MatmulPerfMode.DoubleRow

_BF = ml_dtypes.bfloat16
_F8 = ml_dtypes.float8_e4m3

SW = 32.0                 # fp8 weight pre-scale
SH = 32.0                 # fp8 hidden-state pre-scale
ZS = SW * SH              # PSUM z scale (unwound in the activation)

_cache = {}


def _pack_whT_f8(W):
    """W [4H, K] (K%256==0) -> DoubleRow pack [128, K/128, 4H] fp8 (x SW).

    [p, kt, m] = W.T[p + 128*kt, m]; the DR lhsT for gate m-tile m is
    arr[:, :, 128m : 128m+128].
    """
    M, K = W.shape
    assert K % 256 == 0
    kt = K // 128
    WT = np.ascontiguousarray(W.T) * SW          # [K, M]
    arr = WT.reshape(kt, 128, M).transpose(1, 0, 2)
    return np.ascontiguousarray(arr).astype(_F8)


def _pack_bias(b):
    """b [4H] -> [128, GT] with column m = b[128m:128(m+1)]."""
    return np.ascontiguousarray(b.reshape(GT, 128).T).astype(np.float32)


def _build(T, lstm_bias_flags, repeats=1):
    """Build + compile the per-core program. lstm_bias_flags: 4 bools for
    (enc0, enc1, dec0, dec1) biases being nonzero."""
    nc = bacc.Bacc(
        "TRN2",
        target_bir_lowering=False,
        debug=False,
        enable_asserts=False,
    )

    def din(name, shape, dt):
        return nc.dram_tensor(name, shape, dt, kind="ExternalInput").ap()

    d_xT = din("xT", [IN, S * BC], BF16)
    d_we0x = din("we0x", [IN, 4 * H], BF16)          # x ZS
    d_we0h = din("we0h", [128, HT, 4 * H], F8)       # x SW
    d_we1x = din("we1x", [128, HT, 4 * H], F8)
    d_we1h = din("we1h", [128, HT, 4 * H], F8)
    d_wd0x = din("wd0x", [IN, 4 * H], BF16)          # x ZS
    d_wd0h = din("wd0h", [128, HT, 4 * H], F8)
    d_wd1x = din("wd1x", [128, HT, 4 * H], F8)
    d_wd1h = din("wd1h", [128, HT, 4 * H], F8)
    d_wp1 = din("wp1", [128, HT, HH], F8)
    d_wp2 = din("wp2", [HH, OUT], BF16)
    d_bp1 = din("bp1", [HH, 1], F32)
    d_bp2 = din("bp2", [OUT, 1], F32)
    d_lb = [None] * 4
    for li, flag in enumerate(lstm_bias_flags):
        if flag:
            d_lb[li] = din(f"lstmbias{li}", [128, GT], F32)

    out = nc.dram_tensor("out", [BC, T, OUT], F32, kind="ExternalOutput").ap()
    out_r = out.rearrange("b t c -> c t b")  # [OUT, T, BC] view for DMA scatter

    INV = 1.0 / ZS

    with tile.TileContext(nc) as tc, ExitStack() as ctx:
        persist = ctx.enter_context(tc.tile_pool(name="persist", bufs=1))

        def load(name, dram_ap, shape, dt):
            t = persist.tile(shape, dt, name=name)
            nc.sync.dma_start(t[:], dram_ap[:])
            return t

        s_we0x = load("s_we0x", d_we0x, [IN, 4 * H], BF16)
        s_we0h = load("s_we0h", d_we0h, [128, HT, 4 * H], F8)
        s_we1x = load("s_we1x", d_we1x, [128, HT, 4 * H], F8)
        s_we1h = load("s_we1h", d_we1h, [128, HT, 4 * H], F8)
        s_wd0x = load("s_wd0x", d_wd0x, [IN, 4 * H], BF16)
        s_wd0h = load("s_wd0h", d_wd0h, [128, HT, 4 * H], F8)
        s_wd1x = load("s_wd1x", d_wd1x, [128, HT, 4 * H], F8)
        s_wd1h = load("s_wd1h", d_wd1h, [128, HT, 4 * H], F8)
        s_wp1 = load("s_wp1", d_wp1, [128, HT, HH], F8)
        s_wp2 = load("s_wp2", d_wp2, [HH, OUT], BF16)
        s_bp1 = load("s_bp1", d_bp1, [HH, 1], F32)
        s_bp2 = load("s_bp2", d_bp2, [OUT, 1], F32)
        s_lb = [None] * 4
        for li in range(4):
            if d_lb[li] is not None:
                s_lb[li] = load(f"s_lstmbias{li}", d_lb[li], [128, GT], F32)

        # x.T staged in chunks so step 0 doesn't wait on the whole tensor.
        s_xT = persist.tile([IN, S * BC], BF16, name="s_xT")
        XCH = 8
        chw = S * BC // XCH
        for ci in range(XCH):
            nc.sync.dma_start(
                s_xT[:, ci * chw : (ci + 1) * chw],
                d_xT[:, ci * chw : (ci + 1) * chw],
            )

        zp = ctx.enter_context(tc.tile_pool(name="zp", bufs=3, space="PSUM"))
        mp = ctx.enter_context(tc.tile_pool(name="mp", bufs=1, space="PSUM"))
        hp = ctx.enter_context(tc.tile_pool(name="hp", bufs=1, space="PSUM"))
        gp = ctx.enter_context(tc.tile_pool(name="gp", bufs=2))
        sp = ctx.enter_context(tc.tile_pool(name="sp", bufs=2))

        # HAM heater: the PE's clock gate re-throttles to 1.2 GHz after a
        # ~3.4us idle window. During dependency waits (gates/c/h chain) we
        # issue tiny matmuls whose rhs is the tile just produced by ACT/DVE,
        # so each fires right when that producer finishes -- spaced heartbeats
        # that keep the activity monitor from ever seeing an idle window.
        heat = hp.tile([128, 128], F32, name="heat")
        heat_n = [0]

        def heater(src_ap, cols=64):
            """Tiny matmul reading a freshly-produced [128, >=cols] AP."""
            heat_n[0] += 1
            nc.tensor.matmul(
                heat[0:4, 0:cols],
                s_wp2[:, 0:4],
                src_ap[0:128, 0:cols],
                start=True, stop=True,
            )

        # gate order in z rows: i, f, g, o (PyTorch) -> pair index p below.
        # Emission order f, i, g, o lets DVE start t1 = sig_f * c_prev while
        # the g/o matmuls still stream.
        GATES = (("f", 1, AF.Sigmoid), ("i", 0, AF.Sigmoid),
                 ("g", 2, AF.Tanh), ("o", 3, AF.Sigmoid))

        def cell(tag, layer, h_chunks, x_chunk, c_prev, bias_t, first,
                 x_last=False):
            """Emit one LSTM cell.

            h_chunks: list of (w3d, h3d) DoubleRow K=256 contributions
              (weight tile [128, HT, 4H] fp8, state tile [128, HT, BC] fp8).
            x_chunk: None or (wx, rhs_ap, kpart) bf16 contribution with
              K=kpart (x or pred input; weights pre-scaled by ZS).
            x_last: emit the x contribution after the h ones (decoder: pred
              arrives late, so the h matmuls can run during the MLP chain).
            Returns (h_new[fp8 x SH, 3D], c_new[bf16]).
            """
            n_mm = len(h_chunks) + (1 if x_chunk is not None else 0)
            gate_sb = {}
            for gname, p, func in GATES:
                z = zp.tile([128, HT, BC], F32, tag="z", name=f"z_{tag}_{gname}")
                for j in range(HT):
                    m = 2 * p + j
                    dst = z[:, j, :]

                    def emit_x(mi_):
                        wx, rhs_ap, kpart = x_chunk
                        nc.tensor.matmul(
                            dst, wx[0:kpart, 128 * m : 128 * m + 128], rhs_ap,
                            start=(mi_ == 0), stop=(mi_ == n_mm - 1),
                        )

                    mi = 0
                    if x_chunk is not None and not x_last:
                        emit_x(0)
                        mi = 1
                    for (w3, h3) in h_chunks:
                        nc.tensor.matmul(
                            dst,
                            w3[:, :, 128 * m : 128 * m + 128],
                            h3[:, :, :],
                            start=(mi == 0), stop=(mi == n_mm - 1),
                            perf_mode=DR,
                        )
                        mi += 1
                    if x_chunk is not None and x_last:
                        emit_x(mi)
                g = gp.tile([128, HT, BC], BF16, tag=f"gate_{gname}",
                            name=f"gt_{tag}_{gname}")
                if bias_t is None:
                    nc.scalar.activation(g[:], z[:], func, scale=INV)
                else:
                    for j in range(HT):
                        m = 2 * p + j
                        nc.scalar.activation(
                            g[:, j, :], z[:, j, :], func,
                            bias=bias_t[:, m : m + 1], scale=INV,
                        )
                gate_sb[gname] = g
                heater(g[:, 0, :])

            c_new = sp.tile([128, HT, BC], BF16, tag=f"c{layer}", name=f"c_{tag}")
            h_new = sp.tile([128, HT, BC], F8, tag=f"h{layer}", name=f"h_{tag}")
            tc_t = gp.tile([128, HT, BC], BF16, tag="tanh_c", name=f"tc_{tag}")
            if not first:
                t1 = gp.tile([128, HT, BC], BF16, tag="t1", name=f"t1_{tag}")
                t2 = gp.tile([128, HT, BC], BF16, tag="t2", name=f"t2_{tag}")
            # split the c/tanh chain per hidden half so ACT tanh(half k)
            # overlaps DVE work on half k+1
            for k in range(HT):
                if first:
                    nc.vector.tensor_mul(c_new[:, k, :], gate_sb["i"][:, k, :],
                                         gate_sb["g"][:, k, :])
                else:
                    nc.vector.tensor_mul(t1[:, k, :], gate_sb["f"][:, k, :],
                                         c_prev[:, k, :])
                    nc.vector.tensor_mul(t2[:, k, :], gate_sb["i"][:, k, :],
                                         gate_sb["g"][:, k, :])
                    nc.vector.tensor_add(c_new[:, k, :], t1[:, k, :], t2[:, k, :])
                nc.scalar.activation(tc_t[:, k, :], c_new[:, k, :], AF.Tanh)
                heater(tc_t[:, k, :])
            # h (x SH) in fp8 for the next DoubleRow matmul
            nc.vector.scalar_tensor_tensor(
                h_new[:], gate_sb["o"][:], SH, tc_t[:], ALU.mult, ALU.mult,
            )
            return h_new, c_new

        def emit_forward():
            h0 = c0 = h1 = c1 = None
            # ---------------- encoder ----------------
            for t in range(S):
                first = t == 0
                xt = s_xT[:, BC * t : BC * (t + 1)]
                h0, c0 = cell(
                    f"e0_{t}", 0,
                    [] if first else [(s_we0h, h0)],
                    (s_we0x, xt, IN), c0, s_lb[0], first,
                )
                # h1-part first (older dependency), h0-part last
                h_chunks = ([(s_we1x, h0)] if first
                            else [(s_we1h, h1), (s_we1x, h0)])
                h1, c1 = cell(f"e1_{t}", 1, h_chunks, None, c1, s_lb[1], first)

            # ---------------- decoder ----------------
            pred_bf = None
            for t in range(T):
                if t == 0:
                    xt = s_xT[:, BC * (S - 1) : BC * S]
                else:
                    xt = pred_bf[:]
                # h-part first: those matmuls fill the PE while the previous
                # step's MLP/pred chain is still in flight.
                h0, c0 = cell(
                    f"d0_{t}", 0, [(s_wd0h, h0)], (s_wd0x, xt, IN),
                    c0, s_lb[2], False, x_last=True,
                )
                h1, c1 = cell(
                    f"d1_{t}", 1, [(s_wd1h, h1), (s_wd1x, h0)], None,
                    c1, s_lb[3], False,
                )

                # MLP head: relu(Wp1 @ h1 + bp1) -> sigmoid(Wp2 @ . + bp2)
                m1_ps = mp.tile([HH, BC], F32, tag="m", name=f"m1ps_{t}")
                nc.tensor.matmul(
                    m1_ps[:], s_wp1[:, :, :], h1[:, :, :],
                    start=True, stop=True, perf_mode=DR,
                )
                m1_sb = gp.tile([HH, BC], BF16, tag="m1sb", name=f"m1sb_{t}")
                nc.scalar.activation(m1_sb[:], m1_ps[:], AF.Relu,
                                     bias=s_bp1[:, 0:1], scale=INV)
                m2_ps = mp.tile([OUT, BC], F32, tag="m", name=f"m2ps_{t}")
                nc.tensor.matmul(m2_ps[:], s_wp2[:], m1_sb[:], start=True, stop=True)
                # next-step matmul input first (bf16), then the fp32 DMA copy
                if t < T - 1:
                    pred_bf = gp.tile([OUT, BC], BF16, tag="predbf",
                                      name=f"predbf_{t}")
                    nc.scalar.activation(pred_bf[:], m2_ps[:], AF.Sigmoid,
                                         bias=s_bp2[:, 0:1])
                pred_f = gp.tile([OUT, BC], F32, tag="predf", name=f"predf_{t}")
                nc.scalar.activation(pred_f[:], m2_ps[:], AF.Sigmoid,
                                     bias=s_bp2[:, 0:1])
                nc.sync.dma_start(out_r[:, t, :], pred_f[:])

        for _rep in range(repeats):
            emit_forward()

    nc.compile()
    return nc


def _prep_shared(inputs):
    f32 = lambda k: np.asarray(inputs[k], np.float32)
    shared = {
        "we0x": (np.ascontiguousarray(f32("enc_Wih0").T) * ZS).astype(_BF),
        "we0h": _pack_whT_f8(f32("enc_Whh0")),
        "we1x": _pack_whT_f8(f32("enc_Wih1")),
        "we1h": _pack_whT_f8(f32("enc_Whh1")),
        "wd0x": (np.ascontiguousarray(f32("dec_Wih0").T) * ZS).astype(_BF),
        "wd0h": _pack_whT_f8(f32("dec_Whh0")),
        "wd1x": _pack_whT_f8(f32("dec_Wih1")),
        "wd1h": _pack_whT_f8(f32("dec_Whh1")),
        "wp1": _pack_whT_f8(f32("Wp1")),
        "wp2": np.ascontiguousarray(f32("Wp2").T).astype(_BF),
        "bp1": np.ascontiguousarray(f32("bp1").reshape(HH, 1)),
        "bp2": np.ascontiguousarray(f32("bp2").reshape(OUT, 1)),
    }
    lstm_biases = [f32("enc_b0"), f32("enc_b1"), f32("dec_b0"), f32("dec_b1")]
    flags = tuple(bool(np.any(b != 0)) for b in lstm_biases)
    for li, (b, flag) in enumerate(zip(lstm_biases, flags)):
        if flag:
            shared[f"lstmbias{li}"] = _pack_bias(b)
    return shared, flags


def _make_in_maps(inputs):
    x = np.asarray(inputs["x"], np.float32)
    assert x.shape == (B, S, IN), x.shape
    shared, _ = _prep_shared(inputs)
    in_maps = []
    for c in range(NCORES):
        xc = x[c * BC : (c + 1) * BC]                       # [BC, S, IN]
        xT = np.ascontiguousarray(xc.transpose(2, 1, 0))    # [IN, S, BC]
        in_maps.append({"xT": xT.reshape(IN, S * BC).astype(_BF), **shared})
    return in_maps


def kernel(**inputs):
    T = int(np.asarray(inputs["target_length"]))

    _, flags = _prep_shared(inputs)
    key = (T, flags)
    if key not in _cache:
        _cache[key] = _build(T, flags)
    nc = _cache[key]

    in_maps = _make_in_maps(inputs)

    res = run_bass_kernel_spmd(nc, in_maps, list(range(NCORES)))
    return np.concatenate(
        [res.results[i]["out"] for i in range(NCORES)], axis=0
    ).astype(np.float32)


# revision 10
# speedup vs baseline: 1.0397x; 1.0397x over previous
"""Trainium2 Bass kernel for a seq2seq CandlestickLSTM.

Model (per reference): 2-layer LSTM encoder over S=64 steps, then a
2-layer LSTM decoder run autoregressively for T=32 steps with an MLP
head (Linear(H,H/2) -> ReLU -> Linear(H/2,OUT) -> Sigmoid) whose output
feeds back as the next decoder input.

Sharding: pure data parallel over 8 NeuronCores -- batch 4096 -> 512
rows per core; all weights replicated. No collectives needed.

On-core layout: feature-major ("transposed"): activations h, c live as
[128 partitions, HT, Bc] 3D tiles (hidden tile k = page k). Matmuls
compute z.T = W @ input.T via out = lhsT.T @ rhs with host-pre-packed
bf16 weights (PSUM accumulation fp32).

PE economy: the K=256 hidden projections stream at the N=512 floor
(~216ns/matmul, PE saturated, which also keeps the HAM clock-gate at
2.4 GHz). The K=4 encoder input projections would each waste a full
N-stream on 4/128 rows, so they run as 4 concurrent matmuls packed
into distinct 32-row groups of the PE array via tile_position (x and
its weights replicated at partition bases 0/32/64/96): 8 matmuls in
~2 slot times. fp8 DoubleRow was measured on this hardware at ~429ns
per K=256 matmul -- no faster than two bf16 matmuls -- and the PE
slack it creates causes HAM cold-clock oscillation; bf16 it is.
"""

import numpy as np
import ml_dtypes
from contextlib import ExitStack

import concourse.bass as bass
import concourse.tile as tile
from concourse import bacc, mybir
from concourse.bass_utils import run_bass_kernel_spmd

NCORES = 8
B, S, IN, H, OUT = 4096, 64, 4, 256, 4
BC = B // NCORES          # 512 batch rows per core
HT = H // 128             # 2 hidden 128-tiles
GT = 4 * H // 128         # 8 gate M-tiles
HH = H // 2               # 128 (MLP hidden)
F32 = mybir.dt.float32
BF16 = mybir.dt.bfloat16
AF = mybir.ActivationFunctionType
ALU = mybir.AluOpType

_BF = ml_dtypes.bfloat16

_cache = {}


def _pack_whT(W):
    """W [M, K] (acts on K-dim inputs, K%128==0) -> pack [128, K/128, M].

    [p, kt, m] = W.T[p + 128*kt, m]; the lhsT tile for (k_tile, m_tile) is
    arr[:, kt, 128m : 128m+128].
    """
    M, K = W.shape
    assert K % 128 == 0
    kt = K // 128
    WT = np.ascontiguousarray(W.T)  # [K, M]
    arr = WT.reshape(kt, 128, M).transpose(1, 0, 2)
    return np.ascontiguousarray(arr).astype(_BF)


def _pack_rep(W):
    """W [4H, IN] -> [128, 4H] bf16 with W.T at partition rows 32g..32g+IN-1
    for g in 0..3 (row-group replicas for tile_position-packed matmuls)."""
    rep = np.zeros((128, 4 * H), np.float32)
    WT = W.T  # [IN, 4H]
    for g in range(4):
        rep[32 * g : 32 * g + IN] = WT
    return rep.astype(_BF)


def _pack_bias(b):
    """b [4H] -> [128, GT] with column m = b[128m:128(m+1)]."""
    return np.ascontiguousarray(b.reshape(GT, 128).T).astype(np.float32)


def _build(T, lstm_bias_flags, repeats=1):
    """Build + compile the per-core program. lstm_bias_flags: 4 bools for
    (enc0, enc1, dec0, dec1) biases being nonzero."""
    nc = bacc.Bacc(
        "TRN2",
        target_bir_lowering=False,
        debug=False,
        enable_asserts=False,
    )

    def din(name, shape, dt):
        return nc.dram_tensor(name, shape, dt, kind="ExternalInput").ap()

    d_xT = din("xT", [IN, S * BC], BF16)
    d_we0x = din("we0x", [128, 4 * H], BF16)     # row-group replicated
    d_we0h = din("we0h", [128, HT, 4 * H], BF16)
    d_we1x = din("we1x", [128, HT, 4 * H], BF16)
    d_we1h = din("we1h", [128, HT, 4 * H], BF16)
    d_wd0x = din("wd0x", [IN, 4 * H], BF16)
    d_wd0h = din("wd0h", [128, HT, 4 * H], BF16)
    d_wd1x = din("wd1x", [128, HT, 4 * H], BF16)
    d_wd1h = din("wd1h", [128, HT, 4 * H], BF16)
    d_wp1 = din("wp1", [128, HT, HH], BF16)
    d_wp2 = din("wp2", [HH, OUT], BF16)
    d_bp1 = din("bp1", [HH, 1], F32)
    d_bp2 = din("bp2", [OUT, 1], F32)
    d_lb = [None] * 4
    for li, flag in enumerate(lstm_bias_flags):
        if flag:
            d_lb[li] = din(f"lstmbias{li}", [128, GT], F32)

    out = nc.dram_tensor("out", [BC, T, OUT], F32, kind="ExternalOutput").ap()
    out_r = out.rearrange("b t c -> c t b")  # [OUT, T, BC] view for DMA scatter

    with tile.TileContext(nc) as tc, ExitStack() as ctx:
        persist = ctx.enter_context(tc.tile_pool(name="persist", bufs=1))

        def load(name, dram_ap, shape, dt):
            t = persist.tile(shape, dt, name=name)
            nc.sync.dma_start(t[:], dram_ap[:])
            return t

        s_we0x = load("s_we0x", d_we0x, [128, 4 * H], BF16)
        s_we0h = load("s_we0h", d_we0h, [128, HT, 4 * H], BF16)
        s_we1x = load("s_we1x", d_we1x, [128, HT, 4 * H], BF16)
        s_we1h = load("s_we1h", d_we1h, [128, HT, 4 * H], BF16)
        s_wd0x = load("s_wd0x", d_wd0x, [IN, 4 * H], BF16)
        s_wd0h = load("s_wd0h", d_wd0h, [128, HT, 4 * H], BF16)
        s_wd1x = load("s_wd1x", d_wd1x, [128, HT, 4 * H], BF16)
        s_wd1h = load("s_wd1h", d_wd1h, [128, HT, 4 * H], BF16)
        s_wp1 = load("s_wp1", d_wp1, [128, HT, HH], BF16)
        s_wp2 = load("s_wp2", d_wp2, [HH, OUT], BF16)
        s_bp1 = load("s_bp1", d_bp1, [HH, 1], F32)
        s_bp2 = load("s_bp2", d_bp2, [OUT, 1], F32)
        s_lb = [None] * 4
        for li in range(4):
            if d_lb[li] is not None:
                s_lb[li] = load(f"s_lstmbias{li}", d_lb[li], [128, GT], F32)

        # x.T replicated to partition bases 0/32/64/96 for row-group packing;
        # staged in chunks so step 0 doesn't wait on the whole tensor.
        s_xT = persist.tile([128, S * BC], BF16, name="s_xT")
        XCH = 8
        chw = S * BC // XCH
        for ci in range(XCH):
            for g in range(4):
                nc.sync.dma_start(
                    s_xT[32 * g : 32 * g + IN, ci * chw : (ci + 1) * chw],
                    d_xT[:, ci * chw : (ci + 1) * chw],
                )

        # zeros for the DVE relu (max with 0)
        s_zero = persist.tile([HH, BC], BF16, name="s_zero")
        nc.vector.memset(s_zero[:], 0.0)

        zp = ctx.enter_context(tc.tile_pool(name="zp", bufs=1, space="PSUM"))
        gp = ctx.enter_context(tc.tile_pool(name="gp", bufs=2))
        sp = ctx.enter_context(tc.tile_pool(name="sp", bufs=2))

        # gate order in z rows: i, f, g, o (PyTorch) -> pair index p below.
        # Emission order f, i, g, o lets DVE start t1 = sig_f * c_prev while
        # the g/o matmuls still stream.
        GATES = (("f", 1, AF.Sigmoid), ("i", 0, AF.Sigmoid),
                 ("g", 2, AF.Tanh), ("o", 3, AF.Sigmoid))

        def cell(tag, layer, h_chunks, x_mode, xt, c_prev, bias_t, first):
            """Emit one LSTM cell.

            h_chunks: list of (w3d, h3d, k_first) K=256 contributions; the
              (gate,j) matmul for K-subtile k uses w3d[:, k, 128m:128m+128]
              against h3d[:, k, :].
            x_mode: None | "packed" (4-way tile_position row groups, x first)
              | "plain" (K=IN matmuls at partitions 0..IN-1, emitted last).
            xt: x column block [.., BC] (packed: s_xT cols; plain: any rhs).
            Returns (h_new, c_new) bf16 3D tiles.
            """
            n_h = 2 * len(h_chunks)
            n_mm = n_h + (1 if x_mode else 0)
            z = {}
            for gname, p, func in GATES:
                z[gname] = zp.tile([128, HT, BC], F32, tag=f"z_{gname}", bufs=1,
                                   name=f"z_{tag}_{gname}")

            def x_packed(pair):
                # 4 concurrent matmuls in distinct 32-row groups
                for gi, (gname, p, j) in enumerate(pair):
                    m = 2 * p + j
                    nc.tensor.matmul(
                        z[gname][:, j, :],
                        s_we0x[32 * gi : 32 * gi + IN,
                               128 * m : 128 * m + 128],
                        xt[32 * gi : 32 * gi + IN, :],
                        start=True, stop=(n_mm == 1),
                        tile_position=(32 * gi, 0),
                        skip_group_check=True,
                    )

            if x_mode == "packed":
                x_packed([("f", 1, 0), ("f", 1, 1), ("i", 0, 0), ("i", 0, 1)])

            gate_sb = {}
            for gname, p, func in GATES:
                if x_mode == "packed" and gname == "g":
                    x_packed([("g", 2, 0), ("g", 2, 1),
                              ("o", 3, 0), ("o", 3, 1)])
                for j in range(HT):
                    m = 2 * p + j
                    dst = z[gname][:, j, :]
                    mi = 1 if x_mode == "packed" else 0
                    for (w3, h3) in h_chunks:
                        for k in range(HT):
                            nc.tensor.matmul(
                                dst,
                                w3[:, k, 128 * m : 128 * m + 128],
                                h3[:, k, :],
                                start=(mi == 0),
                                stop=(mi == n_mm - 1),
                                skip_group_check=True,
                            )
                            mi += 1
                    if x_mode == "plain":
                        nc.tensor.matmul(
                            dst, s_wd0x[0:IN, 128 * m : 128 * m + 128], xt,
                            start=(mi == 0), stop=True,
                            skip_group_check=True,
                        )
                g = gp.tile([128, HT, BC], BF16, tag=f"gate_{gname}",
                            name=f"gt_{tag}_{gname}")
                if bias_t is None:
                    nc.scalar.activation(g[:], z[gname][:], func)
                else:
                    for j in range(HT):
                        m = 2 * p + j
                        nc.scalar.activation(
                            g[:, j, :], z[gname][:, j, :], func,
                            bias=bias_t[:, m : m + 1],
                        )
                gate_sb[gname] = g

            c_new = sp.tile([128, HT, BC], BF16, tag=f"c{layer}", name=f"c_{tag}")
            h_new = sp.tile([128, HT, BC], BF16, tag=f"h{layer}", name=f"h_{tag}")
            tc_t = gp.tile([128, HT, BC], BF16, tag="tanh_c", name=f"tc_{tag}")
            if not first:
                t1 = gp.tile([128, HT, BC], BF16, tag="t1", name=f"t1_{tag}")
                t2 = gp.tile([128, HT, BC], BF16, tag="t2", name=f"t2_{tag}")
            for k in range(HT):
                if first:
                    nc.vector.tensor_mul(c_new[:, k, :], gate_sb["i"][:, k, :],
                                         gate_sb["g"][:, k, :])
                else:
                    nc.vector.tensor_mul(t1[:, k, :], gate_sb["f"][:, k, :],
                                         c_prev[:, k, :])
                    nc.vector.tensor_mul(t2[:, k, :], gate_sb["i"][:, k, :],
                                         gate_sb["g"][:, k, :])
                    nc.vector.tensor_add(c_new[:, k, :], t1[:, k, :],
                                         t2[:, k, :])
            nc.scalar.activation(tc_t[:], c_new[:], AF.Tanh)
            for k in range(HT):
                nc.vector.tensor_mul(h_new[:, k, :], gate_sb["o"][:, k, :],
                                     tc_t[:, k, :])
            return h_new, c_new

        def emit_forward():
            h0 = c0 = h1 = c1 = None
            # ---------------- encoder ----------------
            for t in range(S):
                first = t == 0
                xt = s_xT[:, BC * t : BC * (t + 1)]
                h0, c0 = cell(
                    f"e0_{t}", 0,
                    [] if first else [(s_we0h, h0)],
                    "packed", xt, c0, s_lb[0], first,
                )
                h_chunks = ([(s_we1x, h0)] if first
                            else [(s_we1h, h1), (s_we1x, h0)])
                h1, c1 = cell(f"e1_{t}", 1, h_chunks, None, None,
                              c1, s_lb[1], first)

            # ---------------- decoder ----------------
            pred_bf = None
            for t in range(T):
                if t == 0:
                    xt = s_xT[0:IN, BC * (S - 1) : BC * S]
                else:
                    xt = pred_bf[:]
                # h-part first: those matmuls fill the PE while the previous
                # step's MLP/pred chain is still in flight.
                h0, c0 = cell(
                    f"d0_{t}", 0, [(s_wd0h, h0)], "plain", xt,
                    c0, s_lb[2], False,
                )
                h1, c1 = cell(
                    f"d1_{t}", 1, [(s_wd1h, h1), (s_wd1x, h0)], None, None,
                    c1, s_lb[3], False,
                )

                # MLP head: relu(Wp1 @ h1 + bp1) -> sigmoid(Wp2 @ . + bp2)
                m1_ps = zp.tile([HH, BC], F32, tag="z_f", bufs=1, name=f"m1ps_{t}")
                for k in range(HT):
                    nc.tensor.matmul(
                        m1_ps[:], s_wp1[:, k, :], h1[:, k, :],
                        start=(k == 0), stop=(k == HT - 1),
                    )
                # relu on DVE: (m1 + bp1) max 0 -- keeps ACT off the chain
                m1_sb = gp.tile([HH, BC], BF16, tag="m1sb", name=f"m1sb_{t}")
                nc.vector.scalar_tensor_tensor(
                    m1_sb[:], m1_ps[:], s_bp1[:, 0:1], s_zero[:],
                    ALU.add, ALU.max,
                )
                m2_ps = zp.tile([OUT, BC], F32, tag="z_i", bufs=1, name=f"m2ps_{t}")
                nc.tensor.matmul(m2_ps[:], s_wp2[:], m1_sb[:], start=True,
                                 stop=True)
                # next-step matmul input first (bf16), then the fp32 DMA copy
                if t < T - 1:
                    pred_bf = gp.tile([OUT, BC], BF16, tag="predbf",
                                      name=f"predbf_{t}")
                    nc.scalar.activation(pred_bf[:], m2_ps[:], AF.Sigmoid,
                                         bias=s_bp2[:, 0:1])
                pred_f = gp.tile([OUT, BC], F32, tag="predf", name=f"predf_{t}")
                nc.scalar.activation(pred_f[:], m2_ps[:], AF.Sigmoid,
                                     bias=s_bp2[:, 0:1])
                nc.sync.dma_start(out_r[:, t, :], pred_f[:])

        for _rep in range(repeats):
            emit_forward()

    nc.compile()
    return nc


def _prep_shared(inputs):
    f32 = lambda k: np.asarray(inputs[k], np.float32)
    shared = {
        "we0x": _pack_rep(f32("enc_Wih0")),
        "we0h": _pack_whT(f32("enc_Whh0")),
        "we1x": _pack_whT(f32("enc_Wih1")),
        "we1h": _pack_whT(f32("enc_Whh1")),
        "wd0x": np.ascontiguousarray(f32("dec_Wih0").T).astype(_BF),
        "wd0h": _pack_whT(f32("dec_Whh0")),
        "wd1x": _pack_whT(f32("dec_Wih1")),
        "wd1h": _pack_whT(f32("dec_Whh1")),
        "wp1": _pack_whT(f32("Wp1")),
        "wp2": np.ascontiguousarray(f32("Wp2").T).astype(_BF),
        "bp1": np.ascontiguousarray(f32("bp1").reshape(HH, 1)),
        "bp2": np.ascontiguousarray(f32("bp2").reshape(OUT, 1)),
    }
    lstm_biases = [f32("enc_b0"), f32("enc_b1"), f32("dec_b0"), f32("dec_b1")]
    flags = tuple(bool(np.any(b != 0)) for b in lstm_biases)
    for li, (b, flag) in enumerate(zip(lstm_biases, flags)):
        if flag:
            shared[f"lstmbias{li}"] = _pack_bias(b)
    return shared, flags


def _make_in_maps(inputs):
    x = np.asarray(inputs["x"], np.float32)
    assert x.shape == (B, S, IN), x.shape
    shared, _ = _prep_shared(inputs)
    in_maps = []
    for c in range(NCORES):
        xc = x[c * BC : (c + 1) * BC]                       # [BC, S, IN]
        xT = np.ascontiguousarray(xc.transpose(2, 1, 0))    # [IN, S, BC]
        in_maps.append({"xT": xT.reshape(IN, S * BC).astype(_BF), **shared})
    return in_maps


def kernel(**inputs):
    T = int(np.asarray(inputs["target_length"]))

    _, flags = _prep_shared(inputs)
    key = (T, flags)
    if key not in _cache:
        _cache[key] = _build(T, flags)
    nc = _cache[key]

    in_maps = _make_in_maps(inputs)

    res = run_bass_kernel_spmd(nc, in_maps, list(range(NCORES)))
    return np.concatenate(
        [res.results[i]["out"] for i in range(NCORES)], axis=0
    ).astype(np.float32)


# revision 12
# speedup vs baseline: 1.2949x; 1.2454x over previous
"""Trainium2 Bass kernel for a seq2seq CandlestickLSTM.

Model (per reference): 2-layer LSTM encoder over S=64 steps, then a
2-layer LSTM decoder run autoregressively for T=32 steps with an MLP
head (Linear(H,H/2) -> ReLU -> Linear(H/2,OUT) -> Sigmoid) whose output
feeds back as the next decoder input.

Sharding: pure data parallel over 8 NeuronCores -- batch 4096 -> 512
rows per core; all weights replicated. No collectives needed.

On-core layout: feature-major ("transposed"): activations h, c live as
[128 partitions, HT, Bc] 3D tiles (hidden tile k = page k). Matmuls
compute z.T = W @ input.T via out = lhsT.T @ rhs with host-pre-packed
bf16 weights (PSUM accumulation fp32). The PE streams N=512 matmuls at
~216ns back-to-back and is the saturated engine; keeping it saturated
also keeps the HAM clock-gate at 2.4 GHz (fp8 DoubleRow measured ~429ns
per K=256 matmul here -- no faster than two bf16 matmuls -- and the PE
slack it creates causes HAM cold-clock oscillation, so bf16).

Dependency scheduling: within a layer-1 cell (input = h0 just produced
by layer 0), the recurrent Whh@h1 matmuls of ALL four gates are
emitted first so the PE has ~3.5us of ready work while the layer-0
elementwise chain (sig_o -> c -> tanh -> h0) drains; the Wih@h0
matmuls follow per-gate. The same trick fills the decoder's serial
MLP/pred chain with the next step's Whh@h0 matmuls. The elementwise
chain is emitted per hidden half so h[:,k,:] is released to the PE as
soon as its half of tanh(c) and sig_o*tanh(c) complete.
"""

import numpy as np
import ml_dtypes
from contextlib import ExitStack

import concourse.bass as bass
import concourse.tile as tile
from concourse import bacc, mybir
from concourse.bass_utils import run_bass_kernel_spmd

NCORES = 8
B, S, IN, H, OUT = 4096, 64, 4, 256, 4
BC = B // NCORES          # 512 batch rows per core
HT = H // 128             # 2 hidden 128-tiles
GT = 4 * H // 128         # 8 gate M-tiles
HH = H // 2               # 128 (MLP hidden)
F32 = mybir.dt.float32
BF16 = mybir.dt.bfloat16
AF = mybir.ActivationFunctionType
ALU = mybir.AluOpType

_BF = ml_dtypes.bfloat16

_cache = {}


def _pack_whT(W):
    """W [M, K] (acts on K-dim inputs, K%128==0) -> pack [128, K/128, M].

    [p, kt, m] = W.T[p + 128*kt, m]; the lhsT tile for (k_tile, m_tile) is
    arr[:, kt, 128m : 128m+128].
    """
    M, K = W.shape
    assert K % 128 == 0
    kt = K // 128
    WT = np.ascontiguousarray(W.T)  # [K, M]
    arr = WT.reshape(kt, 128, M).transpose(1, 0, 2)
    return np.ascontiguousarray(arr).astype(_BF)


def _pack_bias(b):
    """b [4H] -> [128, GT] with column m = b[128m:128(m+1)]."""
    return np.ascontiguousarray(b.reshape(GT, 128).T).astype(np.float32)


def _build(T, lstm_bias_flags, repeats=1):
    """Build + compile the per-core program. lstm_bias_flags: 4 bools for
    (enc0, enc1, dec0, dec1) biases being nonzero."""
    nc = bacc.Bacc(
        "TRN2",
        target_bir_lowering=False,
        debug=False,
        enable_asserts=False,
    )

    def din(name, shape, dt):
        return nc.dram_tensor(name, shape, dt, kind="ExternalInput").ap()

    d_xT = din("xT", [IN, S * BC], BF16)
    d_we0x = din("we0x", [IN, 4 * H], BF16)
    d_we0h = din("we0h", [128, HT, 4 * H], BF16)
    d_we1x = din("we1x", [128, HT, 4 * H], BF16)
    d_we1h = din("we1h", [128, HT, 4 * H], BF16)
    d_wd0x = din("wd0x", [IN, 4 * H], BF16)
    d_wd0h = din("wd0h", [128, HT, 4 * H], BF16)
    d_wd1x = din("wd1x", [128, HT, 4 * H], BF16)
    d_wd1h = din("wd1h", [128, HT, 4 * H], BF16)
    d_wp1 = din("wp1", [128, HT, HH], BF16)
    d_wp2 = din("wp2", [HH, OUT], BF16)
    d_bp1 = din("bp1", [HH, 1], F32)
    d_bp2 = din("bp2", [OUT, 1], F32)
    d_lb = [None] * 4
    for li, flag in enumerate(lstm_bias_flags):
        if flag:
            d_lb[li] = din(f"lstmbias{li}", [128, GT], F32)

    out = nc.dram_tensor("out", [BC, T, OUT], F32, kind="ExternalOutput").ap()
    out_r = out.rearrange("b t c -> c t b")  # [OUT, T, BC] view for DMA scatter

    with tile.TileContext(nc) as tc, ExitStack() as ctx:
        persist = ctx.enter_context(tc.tile_pool(name="persist", bufs=1))

        def load(name, dram_ap, shape, dt):
            t = persist.tile(shape, dt, name=name)
            nc.sync.dma_start(t[:], dram_ap[:])
            return t

        s_we0x = load("s_we0x", d_we0x, [IN, 4 * H], BF16)
        s_we0h = load("s_we0h", d_we0h, [128, HT, 4 * H], BF16)
        s_we1x = load("s_we1x", d_we1x, [128, HT, 4 * H], BF16)
        s_we1h = load("s_we1h", d_we1h, [128, HT, 4 * H], BF16)
        s_wd0x = load("s_wd0x", d_wd0x, [IN, 4 * H], BF16)
        s_wd0h = load("s_wd0h", d_wd0h, [128, HT, 4 * H], BF16)
        s_wd1x = load("s_wd1x", d_wd1x, [128, HT, 4 * H], BF16)
        s_wd1h = load("s_wd1h", d_wd1h, [128, HT, 4 * H], BF16)
        s_wp1 = load("s_wp1", d_wp1, [128, HT, HH], BF16)
        s_wp2 = load("s_wp2", d_wp2, [HH, OUT], BF16)
        s_bp1 = load("s_bp1", d_bp1, [HH, 1], F32)
        s_bp2 = load("s_bp2", d_bp2, [OUT, 1], F32)
        s_lb = [None] * 4
        for li in range(4):
            if d_lb[li] is not None:
                s_lb[li] = load(f"s_lstmbias{li}", d_lb[li], [128, GT], F32)

        # x.T staged in chunks so step 0 doesn't wait on the whole tensor.
        s_xT = persist.tile([IN, S * BC], BF16, name="s_xT")
        XCH = 8
        chw = S * BC // XCH
        for ci in range(XCH):
            nc.sync.dma_start(
                s_xT[:, ci * chw : (ci + 1) * chw],
                d_xT[:, ci * chw : (ci + 1) * chw],
            )

        # zeros for the DVE relu (max with 0)
        s_zero = persist.tile([HH, BC], BF16, name="s_zero")
        nc.vector.memset(s_zero[:], 0.0)

        zp = ctx.enter_context(tc.tile_pool(name="zp", bufs=1, space="PSUM"))
        gp = ctx.enter_context(tc.tile_pool(name="gp", bufs=2))
        sp = ctx.enter_context(tc.tile_pool(name="sp", bufs=2))

        # gate order in z rows: i, f, g, o (PyTorch) -> pair index p below.
        # Emission order f, i, g, o lets DVE start t1 = sig_f * c_prev while
        # the g/o matmuls still stream.
        GATES = (("f", 1, AF.Sigmoid), ("i", 0, AF.Sigmoid),
                 ("g", 2, AF.Tanh), ("o", 3, AF.Sigmoid))

        def cell(tag, layer, pre_chunks, post_chunks, x_chunk, c_prev,
                 bias_t, first, x_last=False):
            """Emit one LSTM cell.

            pre_chunks: (w3d, h3d) K=256 contributions whose inputs are
              already available -- their matmuls for ALL gates are emitted
              up front (PE fill while upstream elementwise drains).
            post_chunks: contributions on freshly-produced state, emitted
              per-gate after the pre block.
            x_chunk: None or (wx, rhs_ap) K=IN bf16 contribution; emitted
              first unless x_last (decoder: pred arrives late).
            Returns (h_new, c_new) bf16 3D tiles.
            """
            n_mm = 2 * (len(pre_chunks) + len(post_chunks)) + (
                1 if x_chunk is not None else 0)
            z = {}
            for gname, p, func in GATES:
                z[gname] = zp.tile([128, HT, BC], F32, tag=f"z_{gname}",
                                   bufs=1, name=f"z_{tag}_{gname}")

            mi = {(gname, j): 0 for gname, _, _ in GATES for j in range(HT)}

            def emit_x(gname, p, j):
                wx, rhs_ap = x_chunk
                key = (gname, j)
                nc.tensor.matmul(
                    z[gname][:, j, :],
                    wx[0:IN, 128 * (2 * p + j) : 128 * (2 * p + j) + 128],
                    rhs_ap,
                    start=(mi[key] == 0), stop=(mi[key] == n_mm - 1),
                    skip_group_check=True,
                )
                mi[key] += 1

            def emit_h(chunks, gname, p, j):
                key = (gname, j)
                for (w3, h3) in chunks:
                    m = 2 * p + j
                    for k in range(HT):
                        nc.tensor.matmul(
                            z[gname][:, j, :],
                            w3[:, k, 128 * m : 128 * m + 128],
                            h3[:, k, :],
                            start=(mi[key] == 0),
                            stop=(mi[key] == n_mm - 1),
                            skip_group_check=True,
                        )
                        mi[key] += 1

            # pre block: x (if early) and already-available recurrent parts,
            # all gates
            for gname, p, func in GATES:
                if x_chunk is not None and not x_last:
                    emit_x(gname, p, 0)
                    emit_x(gname, p, 1)
                emit_h(pre_chunks, gname, p, 0)
                emit_h(pre_chunks, gname, p, 1)

            # per-gate: fresh-state matmuls + activation
            gate_sb = {}
            for gname, p, func in GATES:
                for j in range(HT):
                    emit_h(post_chunks, gname, p, j)
                    if x_chunk is not None and x_last:
                        emit_x(gname, p, j)
                g = gp.tile([128, HT, BC], BF16, tag=f"gate_{gname}",
                            name=f"gt_{tag}_{gname}")
                if bias_t is None:
                    nc.scalar.activation(g[:], z[gname][:], func)
                else:
                    for j in range(HT):
                        m = 2 * p + j
                        nc.scalar.activation(
                            g[:, j, :], z[gname][:, j, :], func,
                            bias=bias_t[:, m : m + 1],
                        )
                gate_sb[gname] = g

            c_new = sp.tile([128, HT, BC], BF16, tag=f"c{layer}", name=f"c_{tag}")
            h_new = sp.tile([128, HT, BC], BF16, tag=f"h{layer}", name=f"h_{tag}")
            tc_t = gp.tile([128, HT, BC], BF16, tag="tanh_c", name=f"tc_{tag}")
            if not first:
                t1 = gp.tile([128, HT, BC], BF16, tag="t1", name=f"t1_{tag}")
                t2 = gp.tile([128, HT, BC], BF16, tag="t2", name=f"t2_{tag}")
            # full chain per hidden half: half k's h feeds the next cell's
            # K-subtile-k matmuls while half k+1 still streams
            for k in range(HT):
                if first:
                    nc.vector.tensor_mul(c_new[:, k, :], gate_sb["i"][:, k, :],
                                         gate_sb["g"][:, k, :])
                else:
                    nc.vector.tensor_mul(t1[:, k, :], gate_sb["f"][:, k, :],
                                         c_prev[:, k, :])
                    nc.vector.tensor_mul(t2[:, k, :], gate_sb["i"][:, k, :],
                                         gate_sb["g"][:, k, :])
                    nc.vector.tensor_add(c_new[:, k, :], t1[:, k, :],
                                         t2[:, k, :])
                nc.scalar.activation(tc_t[:, k, :], c_new[:, k, :], AF.Tanh)
                nc.vector.tensor_mul(h_new[:, k, :], gate_sb["o"][:, k, :],
                                     tc_t[:, k, :])
            return h_new, c_new

        def emit_forward():
            h0 = c0 = h1 = c1 = None
            # ---------------- encoder ----------------
            for t in range(S):
                first = t == 0
                xt = s_xT[:, BC * t : BC * (t + 1)]
                # L0: x and h0(t-1) both ready at emission -> all pre
                h0, c0 = cell(
                    f"e0_{t}", 0,
                    [] if first else [(s_we0h, h0)], [],
                    (s_we0x, xt), c0, s_lb[0], first,
                )
                # L1: h1(t-1) ready (pre, all gates), h0(t) fresh (post)
                h1, c1 = cell(
                    f"e1_{t}", 1,
                    [] if first else [(s_we1h, h1)], [(s_we1x, h0)],
                    None, c1, s_lb[1], first,
                )

            # ---------------- decoder ----------------
            pred_bf = None
            for t in range(T):
                if t == 0:
                    xt = s_xT[:, BC * (S - 1) : BC * S]
                else:
                    xt = pred_bf[:]
                # L0: h0(t-1) ready (pre -- fills the PE during the previous
                # step's MLP/pred chain), pred arrives late (x_last)
                h0, c0 = cell(
                    f"d0_{t}", 0, [(s_wd0h, h0)], [], (s_wd0x, xt),
                    c0, s_lb[2], False, x_last=True,
                )
                h1, c1 = cell(
                    f"d1_{t}", 1, [(s_wd1h, h1)], [(s_wd1x, h0)], None,
                    c1, s_lb[3], False,
                )

                # MLP head: relu(Wp1 @ h1 + bp1) -> sigmoid(Wp2 @ . + bp2)
                m1_ps = zp.tile([HH, BC], F32, tag="z_f", bufs=1,
                                name=f"m1ps_{t}")
                for k in range(HT):
                    nc.tensor.matmul(
                        m1_ps[:], s_wp1[:, k, :], h1[:, k, :],
                        start=(k == 0), stop=(k == HT - 1),
                    )
                # relu on DVE: (m1 + bp1) max 0 -- keeps ACT off the chain
                m1_sb = gp.tile([HH, BC], BF16, tag="m1sb", name=f"m1sb_{t}")
                nc.vector.scalar_tensor_tensor(
                    m1_sb[:], m1_ps[:], s_bp1[:, 0:1], s_zero[:],
                    ALU.add, ALU.max,
                )
                m2_ps = zp.tile([OUT, BC], F32, tag="z_i", bufs=1,
                                name=f"m2ps_{t}")
                nc.tensor.matmul(m2_ps[:], s_wp2[:], m1_sb[:], start=True,
                                 stop=True)
                # next-step matmul input first (bf16), then the fp32 DMA copy
                if t < T - 1:
                    pred_bf = gp.tile([OUT, BC], BF16, tag="predbf",
                                      name=f"predbf_{t}")
                    nc.scalar.activation(pred_bf[:], m2_ps[:], AF.Sigmoid,
                                         bias=s_bp2[:, 0:1])
                pred_f = gp.tile([OUT, BC], F32, tag="predf", name=f"predf_{t}")
                nc.scalar.activation(pred_f[:], m2_ps[:], AF.Sigmoid,
                                     bias=s_bp2[:, 0:1])
                nc.sync.dma_start(out_r[:, t, :], pred_f[:])

        for _rep in range(repeats):
            emit_forward()

    nc.compile()
    return nc


def _prep_shared(inputs):
    f32 = lambda k: np.asarray(inputs[k], np.float32)
    shared = {
        "we0x": np.ascontiguousarray(f32("enc_Wih0").T).astype(_BF),
        "we0h": _pack_whT(f32("enc_Whh0")),
        "we1x": _pack_whT(f32("enc_Wih1")),
        "we1h": _pack_whT(f32("enc_Whh1")),
        "wd0x": np.ascontiguousarray(f32("dec_Wih0").T).astype(_BF),
        "wd0h": _pack_whT(f32("dec_Whh0")),
        "wd1x": _pack_whT(f32("dec_Wih1")),
        "wd1h": _pack_whT(f32("dec_Whh1")),
        "wp1": _pack_whT(f32("Wp1")),
        "wp2": np.ascontiguousarray(f32("Wp2").T).astype(_BF),
        "bp1": np.ascontiguousarray(f32("bp1").reshape(HH, 1)),
        "bp2": np.ascontiguousarray(f32("bp2").reshape(OUT, 1)),
    }
    lstm_biases = [f32("enc_b0"), f32("enc_b1"), f32("dec_b0"), f32("dec_b1")]
    flags = tuple(bool(np.any(b != 0)) for b in lstm_biases)
    for li, (b, flag) in enumerate(zip(lstm_biases, flags)):
        if flag:
            shared[f"lstmbias{li}"] = _pack_bias(b)
    return shared, flags


def _make_in_maps(inputs):
    x = np.asarray(inputs["x"], np.float32)
    assert x.shape == (B, S, IN), x.shape
    shared, _ = _prep_shared(inputs)
    in_maps = []
    for c in range(NCORES):
        xc = x[c * BC : (c + 1) * BC]                       # [BC, S, IN]
        xT = np.ascontiguousarray(xc.transpose(2, 1, 0))    # [IN, S, BC]
        in_maps.append({"xT": xT.reshape(IN, S * BC).astype(_BF), **shared})
    return in_maps


def kernel(**inputs):
    T = int(np.asarray(inputs["target_length"]))

    _, flags = _prep_shared(inputs)
    key = (T, flags)
    if key not in _cache:
        _cache[key] = _build(T, flags)
    nc = _cache[key]

    in_maps = _make_in_maps(inputs)

    res = run_bass_kernel_spmd(nc, in_maps, list(range(NCORES)))
    return np.concatenate(
        [res.results[i]["out"] for i in range(NCORES)], axis=0
    ).astype(np.float32)


# revision 15
# speedup vs baseline: 1.3073x; 1.0096x over previous
"""Trainium2 Bass kernel for a seq2seq CandlestickLSTM.

Model (per reference): 2-layer LSTM encoder over S=64 steps, then a
2-layer LSTM decoder run autoregressively for T=32 steps with an MLP
head (Linear(H,H/2) -> ReLU -> Linear(H/2,OUT) -> Sigmoid) whose output
feeds back as the next decoder input.

Sharding: pure data parallel over 8 NeuronCores -- batch 4096 -> 512
rows per core; all weights replicated. No collectives needed.

On-core layout: feature-major ("transposed"): activations h, c live as
[128 partitions, HT, Bc] 3D tiles (hidden tile k = page k). Matmuls
compute z.T = W @ input.T via out = lhsT.T @ rhs with host-pre-packed
bf16 weights (PSUM accumulation fp32). The PE streams N=512 matmuls at
~216ns back-to-back and is the saturated engine; keeping it saturated
also keeps the HAM clock-gate at 2.4 GHz (fp8 DoubleRow measured ~429ns
per K=256 matmul here -- no faster than two bf16 matmuls -- and the PE
slack it creates causes HAM cold-clock oscillation, so bf16).

Dependency scheduling: within a layer-1 cell (input = h0 just produced
by layer 0), the recurrent Whh@h1 matmuls of ALL four gates are
emitted first so the PE has ~3.5us of ready work while the layer-0
elementwise chain (sig_o -> c -> tanh -> h0) drains; the Wih@h0
matmuls follow per-gate. The same trick fills the decoder's serial
MLP/pred chain with the next step's Whh@h0 matmuls. The elementwise
chain is emitted per hidden half so h[:,k,:] is released to the PE as
soon as its half of tanh(c) and sig_o*tanh(c) complete.
"""

import numpy as np
import ml_dtypes
from contextlib import ExitStack

import concourse.bass as bass
import concourse.tile as tile
from concourse import bacc, mybir
from concourse.bass_utils import run_bass_kernel_spmd

NCORES = 8
B, S, IN, H, OUT = 4096, 64, 4, 256, 4
BC = B // NCORES          # 512 batch rows per core
HT = H // 128             # 2 hidden 128-tiles
GT = 4 * H // 128         # 8 gate M-tiles
HH = H // 2               # 128 (MLP hidden)
F32 = mybir.dt.float32
BF16 = mybir.dt.bfloat16
AF = mybir.ActivationFunctionType
ALU = mybir.AluOpType

_BF = ml_dtypes.bfloat16

_cache = {}


def _pack_whT(W):
    """W [M, K] (acts on K-dim inputs, K%128==0) -> pack [128, K/128, M].

    [p, kt, m] = W.T[p + 128*kt, m]; the lhsT tile for (k_tile, m_tile) is
    arr[:, kt, 128m : 128m+128].
    """
    M, K = W.shape
    assert K % 128 == 0
    kt = K // 128
    WT = np.ascontiguousarray(W.T)  # [K, M]
    arr = WT.reshape(kt, 128, M).transpose(1, 0, 2)
    return np.ascontiguousarray(arr).astype(_BF)


def _pack_rep(W):
    """W [4H, IN] -> [128, 4H] bf16 with W.T at partition rows 32g..32g+IN-1
    for g in 0..3 (row-group replicas for tile_position-packed matmuls)."""
    rep = np.zeros((128, 4 * H), np.float32)
    for g in range(4):
        rep[32 * g : 32 * g + IN] = W.T
    return rep.astype(_BF)


def _pack_bias_rep(b):
    """b [OUT] -> [128, 1] f32 with b at rows 32g..32g+OUT-1."""
    rep = np.zeros((128, 1), np.float32)
    for g in range(4):
        rep[32 * g : 32 * g + OUT, 0] = b
    return rep


def _pack_bias(b):
    """b [4H] -> [128, GT] with column m = b[128m:128(m+1)]."""
    return np.ascontiguousarray(b.reshape(GT, 128).T).astype(np.float32)


def _build(T, lstm_bias_flags, repeats=1):
    """Build + compile the per-core program. lstm_bias_flags: 4 bools for
    (enc0, enc1, dec0, dec1) biases being nonzero."""
    nc = bacc.Bacc(
        "TRN2",
        target_bir_lowering=False,
        debug=False,
        enable_asserts=False,
    )

    def din(name, shape, dt):
        return nc.dram_tensor(name, shape, dt, kind="ExternalInput").ap()

    d_xT = din("xT", [IN, S * BC], BF16)
    d_we0x = din("we0x", [IN, 4 * H], BF16)
    d_we0h = din("we0h", [128, HT, 4 * H], BF16)
    d_we1x = din("we1x", [128, HT, 4 * H], BF16)
    d_we1h = din("we1h", [128, HT, 4 * H], BF16)
    d_wd0x = din("wd0x", [128, 4 * H], BF16)   # row-group replicated
    d_wd0h = din("wd0h", [128, HT, 4 * H], BF16)
    d_wd1x = din("wd1x", [128, HT, 4 * H], BF16)
    d_wd1h = din("wd1h", [128, HT, 4 * H], BF16)
    d_wp1 = din("wp1", [128, HT, HH], BF16)
    d_wp2 = din("wp2", [HH, OUT], BF16)
    d_bp1 = din("bp1", [HH, 1], F32)
    d_bp2 = din("bp2", [128, 1], F32)          # row-group replicated
    d_lb = [None] * 4
    for li, flag in enumerate(lstm_bias_flags):
        if flag:
            d_lb[li] = din(f"lstmbias{li}", [128, GT], F32)

    out = nc.dram_tensor("out", [BC, T, OUT], F32, kind="ExternalOutput").ap()
    out_r = out.rearrange("b t c -> c t b")  # [OUT, T, BC] view for DMA scatter

    with tile.TileContext(nc) as tc, ExitStack() as ctx:
        persist = ctx.enter_context(tc.tile_pool(name="persist", bufs=1))

        def load(name, dram_ap, shape, dt):
            t = persist.tile(shape, dt, name=name)
            nc.sync.dma_start(t[:], dram_ap[:])
            return t

        s_we0x = load("s_we0x", d_we0x, [IN, 4 * H], BF16)
        s_we0h = load("s_we0h", d_we0h, [128, HT, 4 * H], BF16)
        s_we1x = load("s_we1x", d_we1x, [128, HT, 4 * H], BF16)
        s_we1h = load("s_we1h", d_we1h, [128, HT, 4 * H], BF16)
        s_wd0x = load("s_wd0x", d_wd0x, [128, 4 * H], BF16)
        s_wd0h = load("s_wd0h", d_wd0h, [128, HT, 4 * H], BF16)
        s_wd1x = load("s_wd1x", d_wd1x, [128, HT, 4 * H], BF16)
        s_wd1h = load("s_wd1h", d_wd1h, [128, HT, 4 * H], BF16)
        s_wp1 = load("s_wp1", d_wp1, [128, HT, HH], BF16)
        s_wp2 = load("s_wp2", d_wp2, [HH, OUT], BF16)
        s_bp1 = load("s_bp1", d_bp1, [HH, 1], F32)
        s_bp2 = load("s_bp2", d_bp2, [128, 1], F32)
        s_lb = [None] * 4
        for li in range(4):
            if d_lb[li] is not None:
                s_lb[li] = load(f"s_lstmbias{li}", d_lb[li], [128, GT], F32)

        # x.T staged in chunks so step 0 doesn't wait on the whole tensor.
        s_xT = persist.tile([IN, S * BC], BF16, name="s_xT")
        XCH = 8
        chw = S * BC // XCH
        for ci in range(XCH):
            nc.sync.dma_start(
                s_xT[:, ci * chw : (ci + 1) * chw],
                d_xT[:, ci * chw : (ci + 1) * chw],
            )

        # zeros for the DVE relu (max with 0)
        s_zero = persist.tile([HH, BC], BF16, name="s_zero")
        nc.vector.memset(s_zero[:], 0.0)

        zp = ctx.enter_context(tc.tile_pool(name="zp", bufs=1, space="PSUM"))
        gp = ctx.enter_context(tc.tile_pool(name="gp", bufs=2))
        sp = ctx.enter_context(tc.tile_pool(name="sp", bufs=2))

        # gate order in z rows: i, f, g, o (PyTorch) -> pair index p below.
        # Emission order f, i, g, o lets DVE start t1 = sig_f * c_prev while
        # the g/o matmuls still stream.
        GATES = (("f", 1, AF.Sigmoid), ("i", 0, AF.Sigmoid),
                 ("g", 2, AF.Tanh), ("o", 3, AF.Sigmoid))

        def cell(tag, layer, pre_chunks, post_chunks, x_chunk, c_prev,
                 bias_t, first, x_mode="first"):
            """Emit one LSTM cell.

            pre_chunks: (w3d, h3d) K=256 contributions whose inputs are
              already available -- their matmuls for ALL gates are emitted
              up front (PE fill while upstream elementwise drains).
            post_chunks: contributions on freshly-produced state, emitted
              per-gate after the pre block.
            x_chunk: None or (wx, rhs_ap) K=IN bf16 contribution.
            x_mode: "first" (emit in the pre block) | "last" (per-gate,
              after post -- decoder step 0) | "packed_last" (4-way
              tile_position row groups per gate quad; wx/rhs replicated
              at partition bases 0/32/64/96).
            Returns (h_new, c_new) bf16 3D tiles.
            """
            n_mm = 2 * (len(pre_chunks) + len(post_chunks)) + (
                1 if x_chunk is not None else 0)
            z = {}
            for gname, p, func in GATES:
                z[gname] = zp.tile([128, HT, BC], F32, tag=f"z_{gname}",
                                   bufs=1, name=f"z_{tag}_{gname}")

            mi = {(gname, j): 0 for gname, _, _ in GATES for j in range(HT)}

            def emit_x(gname, p, j, base=0, tile_pos=None):
                wx, rhs_ap = x_chunk
                key = (gname, j)
                m = 2 * p + j
                nc.tensor.matmul(
                    z[gname][:, j, :],
                    wx[base : base + IN, 128 * m : 128 * m + 128],
                    rhs_ap[base : base + IN, :],
                    start=(mi[key] == 0), stop=(mi[key] == n_mm - 1),
                    tile_position=tile_pos,
                    skip_group_check=True,
                )
                mi[key] += 1

            def emit_x_packed(quad):
                # 4 concurrent matmuls in distinct 32-row groups (x/pred and
                # weights replicated at partition bases 0/32/64/96)
                for gi, (gname, p, j) in enumerate(quad):
                    emit_x(gname, p, j, base=32 * gi, tile_pos=(32 * gi, 0))

            def emit_h(chunks, gname, p, j):
                key = (gname, j)
                for (w3, h3) in chunks:
                    m = 2 * p + j
                    for k in range(HT):
                        nc.tensor.matmul(
                            z[gname][:, j, :],
                            w3[:, k, 128 * m : 128 * m + 128],
                            h3[:, k, :],
                            start=(mi[key] == 0),
                            stop=(mi[key] == n_mm - 1),
                            skip_group_check=True,
                        )
                        mi[key] += 1

            # pre block: x (if early) and already-available recurrent parts,
            # all gates
            for gname, p, func in GATES:
                if x_chunk is not None and x_mode == "first":
                    emit_x(gname, p, 0)
                    emit_x(gname, p, 1)
                emit_h(pre_chunks, gname, p, 0)
                emit_h(pre_chunks, gname, p, 1)

            # per-gate: fresh-state matmuls + activation
            gate_sb = {}
            for gname, p, func in GATES:
                if x_mode == "packed_last":
                    if gname == "f":
                        emit_x_packed([("f", 1, 0), ("f", 1, 1),
                                       ("i", 0, 0), ("i", 0, 1)])
                    elif gname == "g":
                        emit_x_packed([("g", 2, 0), ("g", 2, 1),
                                       ("o", 3, 0), ("o", 3, 1)])
                for j in range(HT):
                    emit_h(post_chunks, gname, p, j)
                    if x_chunk is not None and x_mode == "last":
                        emit_x(gname, p, j)
                g = gp.tile([128, HT, BC], BF16, tag=f"gate_{gname}",
                            name=f"gt_{tag}_{gname}")
                if bias_t is None:
                    nc.scalar.activation(g[:], z[gname][:], func)
                else:
                    for j in range(HT):
                        m = 2 * p + j
                        nc.scalar.activation(
                            g[:, j, :], z[gname][:, j, :], func,
                            bias=bias_t[:, m : m + 1],
                        )
                gate_sb[gname] = g

            c_new = sp.tile([128, HT, BC], BF16, tag=f"c{layer}", name=f"c_{tag}")
            h_new = sp.tile([128, HT, BC], BF16, tag=f"h{layer}", name=f"h_{tag}")
            tc_t = gp.tile([128, HT, BC], BF16, tag="tanh_c", name=f"tc_{tag}")
            if not first:
                t1 = gp.tile([128, HT, BC], BF16, tag="t1", name=f"t1_{tag}")
                t2 = gp.tile([128, HT, BC], BF16, tag="t2", name=f"t2_{tag}")
            # full chain per hidden half: half k's h feeds the next cell's
            # K-subtile-k matmuls while half k+1 still streams
            for k in range(HT):
                if first:
                    nc.vector.tensor_mul(c_new[:, k, :], gate_sb["i"][:, k, :],
                                         gate_sb["g"][:, k, :])
                else:
                    nc.vector.tensor_mul(t1[:, k, :], gate_sb["f"][:, k, :],
                                         c_prev[:, k, :])
                    nc.vector.tensor_mul(t2[:, k, :], gate_sb["i"][:, k, :],
                                         gate_sb["g"][:, k, :])
                    nc.vector.tensor_add(c_new[:, k, :], t1[:, k, :],
                                         t2[:, k, :])
                nc.scalar.activation(tc_t[:, k, :], c_new[:, k, :], AF.Tanh)
                nc.vector.tensor_mul(h_new[:, k, :], gate_sb["o"][:, k, :],
                                     tc_t[:, k, :])
            return h_new, c_new

        def emit_forward():
            h0 = c0 = h1 = c1 = None
            # ---------------- encoder ----------------
            for t in range(S):
                first = t == 0
                xt = s_xT[:, BC * t : BC * (t + 1)]
                # L0: x and h0(t-1) both ready at emission -> all pre
                h0, c0 = cell(
                    f"e0_{t}", 0,
                    [] if first else [(s_we0h, h0)], [],
                    (s_we0x, xt), c0, s_lb[0], first,
                )
                # L1: h1(t-1) ready (pre, all gates), h0(t) fresh (post)
                h1, c1 = cell(
                    f"e1_{t}", 1,
                    [] if first else [(s_we1h, h1)], [(s_we1x, h0)],
                    None, c1, s_lb[1], first,
                )

            # ---------------- decoder ----------------
            pred_bf = None
            for t in range(T):
                if t == 0:
                    xt = s_xT[:, BC * (S - 1) : BC * S]
                else:
                    xt = pred_bf[:]
                # L0: h0(t-1) ready (pre -- fills the PE during the previous
                # step's MLP/pred chain), pred arrives late (emitted last)
                h0, c0 = cell(
                    f"d0_{t}", 0, [(s_wd0h, h0)], [], (s_wd0x, xt),
                    c0, s_lb[2], False,
                    x_mode=("last" if t == 0 else "packed_last"),
                )
                h1, c1 = cell(
                    f"d1_{t}", 1, [(s_wd1h, h1)], [(s_wd1x, h0)], None,
                    c1, s_lb[3], False,
                )

                # MLP head: relu(Wp1 @ h1 + bp1) -> sigmoid(Wp2 @ . + bp2)
                m1_ps = zp.tile([HH, BC], F32, tag="z_f", bufs=1,
                                name=f"m1ps_{t}")
                for k in range(HT):
                    nc.tensor.matmul(
                        m1_ps[:], s_wp1[:, k, :], h1[:, k, :],
                        start=(k == 0), stop=(k == HT - 1),
                    )
                # relu on DVE: (m1 + bp1) max 0 -- keeps ACT off the chain
                m1_sb = gp.tile([HH, BC], BF16, tag="m1sb", name=f"m1sb_{t}")
                nc.vector.scalar_tensor_tensor(
                    m1_sb[:], m1_ps[:], s_bp1[:, 0:1], s_zero[:],
                    ALU.add, ALU.max,
                )
                # m2 as 4 concurrent col-group matmuls: pred lands at
                # partition bases 0/32/64/96 so the next step's packed L0
                # input matmuls can read their row-group replicas directly.
                m2_ps = zp.tile([128, BC], F32, tag="z_i", bufs=1,
                                name=f"m2ps_{t}")
                for g in range(4):
                    nc.tensor.matmul(
                        m2_ps[32 * g : 32 * g + OUT, :], s_wp2[:],
                        m1_sb[:], start=True, stop=True,
                        tile_position=(0, 32 * g),
                        skip_group_check=True,
                    )
                # next-step matmul input first (bf16, all four replicas in
                # one activation -- partitions are free parallelism), then
                # the fp32 DMA copy
                if t < T - 1:
                    pred_bf = gp.tile([128, BC], BF16, tag="predbf",
                                      name=f"predbf_{t}")
                    nc.scalar.activation(pred_bf[:], m2_ps[:], AF.Sigmoid,
                                         bias=s_bp2[:, 0:1])
                pred_f = gp.tile([OUT, BC], F32, tag="predf", name=f"predf_{t}")
                nc.scalar.activation(pred_f[:], m2_ps[0:OUT, :], AF.Sigmoid,
                                     bias=s_bp2[0:OUT, 0:1])
                nc.sync.dma_start(out_r[:, t, :], pred_f[:])

        for _rep in range(repeats):
            emit_forward()

    nc.compile()
    return nc


def _prep_shared(inputs):
    f32 = lambda k: np.asarray(inputs[k], np.float32)
    shared = {
        "we0x": np.ascontiguousarray(f32("enc_Wih0").T).astype(_BF),
        "we0h": _pack_whT(f32("enc_Whh0")),
        "we1x": _pack_whT(f32("enc_Wih1")),
        "we1h": _pack_whT(f32("enc_Whh1")),
        "wd0x": _pack_rep(f32("dec_Wih0")),
        "wd0h": _pack_whT(f32("dec_Whh0")),
        "wd1x": _pack_whT(f32("dec_Wih1")),
        "wd1h": _pack_whT(f32("dec_Whh1")),
        "wp1": _pack_whT(f32("Wp1")),
        "wp2": np.ascontiguousarray(f32("Wp2").T).astype(_BF),
        "bp1": np.ascontiguousarray(f32("bp1").reshape(HH, 1)),
        "bp2": _pack_bias_rep(f32("bp2").reshape(OUT)),
    }
    lstm_biases = [f32("enc_b0"), f32("enc_b1"), f32("dec_b0"), f32("dec_b1")]
    flags = tuple(bool(np.any(b != 0)) for b in lstm_biases)
    for li, (b, flag) in enumerate(zip(lstm_biases, flags)):
        if flag:
            shared[f"lstmbias{li}"] = _pack_bias(b)
    return shared, flags


def _make_in_maps(inputs):
    x = np.asarray(inputs["x"], np.float32)
    assert x.shape == (B, S, IN), x.shape
    shared, _ = _prep_shared(inputs)
    in_maps = []
    for c in range(NCORES):
        xc = x[c * BC : (c + 1) * BC]                       # [BC, S, IN]
        xT = np.ascontiguousarray(xc.transpose(2, 1, 0))    # [IN, S, BC]
        in_maps.append({"xT": xT.reshape(IN, S * BC).astype(_BF), **shared})
    return in_maps


def kernel(**inputs):
    T = int(np.asarray(inputs["target_length"]))

    _, flags = _prep_shared(inputs)
    key = (T, flags)
    if key not in _cache:
        _cache[key] = _build(T, flags)
    nc = _cache[key]

    in_maps = _make_in_maps(inputs)

    res = run_bass_kernel_spmd(nc, in_maps, list(range(NCORES)))
    return np.concatenate(
        [res.results[i]["out"] for i in range(NCORES)], axis=0
    ).astype(np.float32)


# revision 17
# speedup vs baseline: 1.3073x; 1.0000x over previous
"""Trainium2 Bass kernel for a seq2seq CandlestickLSTM.

Model (per reference): 2-layer LSTM encoder over S=64 steps, then a
2-layer LSTM decoder run autoregressively for T=32 steps with an MLP
head (Linear(H,H/2) -> ReLU -> Linear(H/2,OUT) -> Sigmoid) whose output
feeds back as the next decoder input.

Sharding: pure data parallel over 8 NeuronCores -- batch 4096 -> 512
rows per core; all weights replicated. No collectives needed.

On-core layout: feature-major ("transposed"): activations h, c live as
[128 partitions, HT, Bc] 3D tiles (hidden tile k = page k). Matmuls
compute z.T = W @ input.T via out = lhsT.T @ rhs with host-pre-packed
bf16 weights (PSUM accumulation fp32). The PE streams N=512 matmuls at
~216ns back-to-back and is the saturated engine; keeping it saturated
also keeps the HAM clock-gate at 2.4 GHz (fp8 DoubleRow measured ~429ns
per K=256 matmul here -- no faster than two bf16 matmuls -- and the PE
slack it creates causes HAM cold-clock oscillation, so bf16).

Dependency scheduling: within a layer-1 cell (input = h0 just produced
by layer 0), the recurrent Whh@h1 matmuls of ALL four gates are
emitted first so the PE has ~3.5us of ready work while the layer-0
elementwise chain (sig_o -> c -> tanh -> h0) drains; the Wih@h0
matmuls follow per-gate. The same trick fills the decoder's serial
MLP/pred chain with the next step's Whh@h0 matmuls. The elementwise
chain is emitted per hidden half so h[:,k,:] is released to the PE as
soon as its half of tanh(c) and sig_o*tanh(c) complete.
"""

import numpy as np
import ml_dtypes
from contextlib import ExitStack

import concourse.bass as bass
import concourse.tile as tile
from concourse import bacc, mybir
from concourse.bass_utils import run_bass_kernel_spmd

NCORES = 8
B, S, IN, H, OUT = 4096, 64, 4, 256, 4
BC = B // NCORES          # 512 batch rows per core
HT = H // 128             # 2 hidden 128-tiles
GT = 4 * H // 128         # 8 gate M-tiles
HH = H // 2               # 128 (MLP hidden)
F32 = mybir.dt.float32
BF16 = mybir.dt.bfloat16
AF = mybir.ActivationFunctionType
ALU = mybir.AluOpType

_BF = ml_dtypes.bfloat16

_cache = {}


def _pack_whT(W):
    """W [M, K] (acts on K-dim inputs, K%128==0) -> pack [128, K/128, M].

    [p, kt, m] = W.T[p + 128*kt, m]; the lhsT tile for (k_tile, m_tile) is
    arr[:, kt, 128m : 128m+128].
    """
    M, K = W.shape
    assert K % 128 == 0
    kt = K // 128
    WT = np.ascontiguousarray(W.T)  # [K, M]
    arr = WT.reshape(kt, 128, M).transpose(1, 0, 2)
    return np.ascontiguousarray(arr).astype(_BF)


def _pack_rep(W):
    """W [4H, IN] -> [128, 4H] bf16 with W.T at partition rows 32g..32g+IN-1
    for g in 0..3 (row-group replicas for tile_position-packed matmuls)."""
    rep = np.zeros((128, 4 * H), np.float32)
    for g in range(4):
        rep[32 * g : 32 * g + IN] = W.T
    return rep.astype(_BF)


def _pack_bias_rep(b):
    """b [OUT] -> [128, 1] f32 with b at rows 32g..32g+OUT-1."""
    rep = np.zeros((128, 1), np.float32)
    for g in range(4):
        rep[32 * g : 32 * g + OUT, 0] = b
    return rep


def _pack_bias(b):
    """b [4H] -> [128, GT] with column m = b[128m:128(m+1)]."""
    return np.ascontiguousarray(b.reshape(GT, 128).T).astype(np.float32)


def _build(T, lstm_bias_flags, repeats=1):
    """Build + compile the per-core program. lstm_bias_flags: 4 bools for
    (enc0, enc1, dec0, dec1) biases being nonzero."""
    nc = bacc.Bacc(
        "TRN2",
        target_bir_lowering=False,
        debug=False,
        enable_asserts=False,
    )

    def din(name, shape, dt):
        return nc.dram_tensor(name, shape, dt, kind="ExternalInput").ap()

    d_xT = din("xT", [IN, S * BC], BF16)
    d_we0x = din("we0x", [IN, 4 * H], BF16)
    d_we0h = din("we0h", [128, HT, 4 * H], BF16)
    d_we1x = din("we1x", [128, HT, 4 * H], BF16)
    d_we1h = din("we1h", [128, HT, 4 * H], BF16)
    d_wd0x = din("wd0x", [128, 4 * H], BF16)   # row-group replicated
    d_wd0h = din("wd0h", [128, HT, 4 * H], BF16)
    d_wd1x = din("wd1x", [128, HT, 4 * H], BF16)
    d_wd1h = din("wd1h", [128, HT, 4 * H], BF16)
    d_wp1 = din("wp1", [128, HT, HH], BF16)
    d_wp2 = din("wp2", [HH, OUT], BF16)
    d_bp1 = din("bp1", [HH, 1], F32)
    d_bp2 = din("bp2", [128, 1], F32)          # row-group replicated
    d_lb = [None] * 4
    for li, flag in enumerate(lstm_bias_flags):
        if flag:
            d_lb[li] = din(f"lstmbias{li}", [128, GT], F32)

    out = nc.dram_tensor("out", [BC, T, OUT], F32, kind="ExternalOutput").ap()
    out_r = out.rearrange("b t c -> c t b")  # [OUT, T, BC] view for DMA scatter

    with tile.TileContext(nc) as tc, ExitStack() as ctx:
        persist = ctx.enter_context(tc.tile_pool(name="persist", bufs=1))

        def load(name, dram_ap, shape, dt):
            t = persist.tile(shape, dt, name=name)
            nc.sync.dma_start(t[:], dram_ap[:])
            return t

        s_we0x = load("s_we0x", d_we0x, [IN, 4 * H], BF16)
        s_we0h = load("s_we0h", d_we0h, [128, HT, 4 * H], BF16)
        s_we1x = load("s_we1x", d_we1x, [128, HT, 4 * H], BF16)
        s_we1h = load("s_we1h", d_we1h, [128, HT, 4 * H], BF16)
        s_wd0x = load("s_wd0x", d_wd0x, [128, 4 * H], BF16)
        s_wd0h = load("s_wd0h", d_wd0h, [128, HT, 4 * H], BF16)
        s_wd1x = load("s_wd1x", d_wd1x, [128, HT, 4 * H], BF16)
        s_wd1h = load("s_wd1h", d_wd1h, [128, HT, 4 * H], BF16)
        s_wp1 = load("s_wp1", d_wp1, [128, HT, HH], BF16)
        s_wp2 = load("s_wp2", d_wp2, [HH, OUT], BF16)
        s_bp1 = load("s_bp1", d_bp1, [HH, 1], F32)
        s_bp2 = load("s_bp2", d_bp2, [128, 1], F32)
        s_lb = [None] * 4
        for li in range(4):
            if d_lb[li] is not None:
                s_lb[li] = load(f"s_lstmbias{li}", d_lb[li], [128, GT], F32)

        # x.T staged in chunks so step 0 doesn't wait on the whole tensor.
        s_xT = persist.tile([IN, S * BC], BF16, name="s_xT")
        XCH = 8
        chw = S * BC // XCH
        for ci in range(XCH):
            nc.sync.dma_start(
                s_xT[:, ci * chw : (ci + 1) * chw],
                d_xT[:, ci * chw : (ci + 1) * chw],
            )

        # zeros for the DVE relu (max with 0)
        s_zero = persist.tile([HH, BC], BF16, name="s_zero")
        nc.vector.memset(s_zero[:], 0.0)

        zp = ctx.enter_context(tc.tile_pool(name="zp", bufs=1, space="PSUM"))
        gp = ctx.enter_context(tc.tile_pool(name="gp", bufs=3))
        sp = ctx.enter_context(tc.tile_pool(name="sp", bufs=3))

        # gate order in z rows: i, f, g, o (PyTorch) -> pair index p below.
        # Emission order f, i, g, o lets DVE start t1 = sig_f * c_prev while
        # the g/o matmuls still stream.
        GATES = (("f", 1, AF.Sigmoid), ("i", 0, AF.Sigmoid),
                 ("g", 2, AF.Tanh), ("o", 3, AF.Sigmoid))

        def cell(tag, layer, pre_chunks, post_chunks, x_chunk, c_prev,
                 bias_t, first, x_mode="first"):
            """Emit one LSTM cell.

            pre_chunks: (w3d, h3d) K=256 contributions whose inputs are
              already available -- their matmuls for ALL gates are emitted
              up front (PE fill while upstream elementwise drains).
            post_chunks: contributions on freshly-produced state, emitted
              per-gate after the pre block.
            x_chunk: None or (wx, rhs_ap) K=IN bf16 contribution.
            x_mode: "first" (emit in the pre block) | "last" (per-gate,
              after post -- decoder step 0) | "packed_last" (4-way
              tile_position row groups per gate quad; wx/rhs replicated
              at partition bases 0/32/64/96).
            Returns (h_new, c_new) bf16 3D tiles.
            """
            n_mm = 2 * (len(pre_chunks) + len(post_chunks)) + (
                1 if x_chunk is not None else 0)
            z = {}
            for gname, p, func in GATES:
                z[gname] = zp.tile([128, HT, BC], F32, tag=f"z_{gname}",
                                   bufs=1, name=f"z_{tag}_{gname}")

            mi = {(gname, j): 0 for gname, _, _ in GATES for j in range(HT)}

            def emit_x(gname, p, j, base=0, tile_pos=None):
                wx, rhs_ap = x_chunk
                key = (gname, j)
                m = 2 * p + j
                nc.tensor.matmul(
                    z[gname][:, j, :],
                    wx[base : base + IN, 128 * m : 128 * m + 128],
                    rhs_ap[base : base + IN, :],
                    start=(mi[key] == 0), stop=(mi[key] == n_mm - 1),
                    tile_position=tile_pos,
                    skip_group_check=True,
                )
                mi[key] += 1

            def emit_x_packed(quad):
                # 4 concurrent matmuls in distinct 32-row groups (x/pred and
                # weights replicated at partition bases 0/32/64/96)
                for gi, (gname, p, j) in enumerate(quad):
                    emit_x(gname, p, j, base=32 * gi, tile_pos=(32 * gi, 0))

            def emit_h(chunks, gname, p, j):
                key = (gname, j)
                for (w3, h3) in chunks:
                    m = 2 * p + j
                    for k in range(HT):
                        nc.tensor.matmul(
                            z[gname][:, j, :],
                            w3[:, k, 128 * m : 128 * m + 128],
                            h3[:, k, :],
                            start=(mi[key] == 0),
                            stop=(mi[key] == n_mm - 1),
                            skip_group_check=True,
                        )
                        mi[key] += 1

            # pre block: x (if early) and already-available recurrent parts,
            # all gates
            for gname, p, func in GATES:
                if x_chunk is not None and x_mode == "first":
                    emit_x(gname, p, 0)
                    emit_x(gname, p, 1)
                emit_h(pre_chunks, gname, p, 0)
                emit_h(pre_chunks, gname, p, 1)

            # per-gate: fresh-state matmuls + activation
            gate_sb = {}
            for gname, p, func in GATES:
                if x_mode == "packed_last":
                    if gname == "f":
                        emit_x_packed([("f", 1, 0), ("f", 1, 1),
                                       ("i", 0, 0), ("i", 0, 1)])
                    elif gname == "g":
                        emit_x_packed([("g", 2, 0), ("g", 2, 1),
                                       ("o", 3, 0), ("o", 3, 1)])
                for j in range(HT):
                    emit_h(post_chunks, gname, p, j)
                    if x_chunk is not None and x_mode == "last":
                        emit_x(gname, p, j)
                g = gp.tile([128, HT, BC], BF16, tag=f"gate_{gname}",
                            name=f"gt_{tag}_{gname}")
                if bias_t is None:
                    nc.scalar.activation(g[:], z[gname][:], func)
                else:
                    for j in range(HT):
                        m = 2 * p + j
                        nc.scalar.activation(
                            g[:, j, :], z[gname][:, j, :], func,
                            bias=bias_t[:, m : m + 1],
                        )
                gate_sb[gname] = g

            c_new = sp.tile([128, HT, BC], BF16, tag=f"c{layer}", name=f"c_{tag}")
            h_new = sp.tile([128, HT, BC], BF16, tag=f"h{layer}", name=f"h_{tag}")
            tc_t = gp.tile([128, HT, BC], BF16, tag="tanh_c", name=f"tc_{tag}")
            if not first:
                t1 = gp.tile([128, HT, BC], BF16, tag="t1", name=f"t1_{tag}")
                t2 = gp.tile([128, HT, BC], BF16, tag="t2", name=f"t2_{tag}")
            # full chain per hidden half: half k's h feeds the next cell's
            # K-subtile-k matmuls while half k+1 still streams
            for k in range(HT):
                if first:
                    nc.vector.tensor_mul(c_new[:, k, :], gate_sb["i"][:, k, :],
                                         gate_sb["g"][:, k, :])
                else:
                    nc.vector.tensor_mul(t1[:, k, :], gate_sb["f"][:, k, :],
                                         c_prev[:, k, :])
                    nc.vector.tensor_mul(t2[:, k, :], gate_sb["i"][:, k, :],
                                         gate_sb["g"][:, k, :])
                    nc.vector.tensor_add(c_new[:, k, :], t1[:, k, :],
                                         t2[:, k, :])
                nc.scalar.activation(tc_t[:, k, :], c_new[:, k, :], AF.Tanh)
                nc.vector.tensor_mul(h_new[:, k, :], gate_sb["o"][:, k, :],
                                     tc_t[:, k, :])
            return h_new, c_new

        def emit_forward():
            h0 = c0 = h1 = c1 = None
            # ---------------- encoder ----------------
            for t in range(S):
                first = t == 0
                xt = s_xT[:, BC * t : BC * (t + 1)]
                # L0: x and h0(t-1) both ready at emission -> all pre
                h0, c0 = cell(
                    f"e0_{t}", 0,
                    [] if first else [(s_we0h, h0)], [],
                    (s_we0x, xt), c0, s_lb[0], first,
                )
                # L1: h1(t-1) ready (pre, all gates), h0(t) fresh (post)
                h1, c1 = cell(
                    f"e1_{t}", 1,
                    [] if first else [(s_we1h, h1)], [(s_we1x, h0)],
                    None, c1, s_lb[1], first,
                )

            # ---------------- decoder ----------------
            pred_bf = None
            for t in range(T):
                if t == 0:
                    xt = s_xT[:, BC * (S - 1) : BC * S]
                else:
                    xt = pred_bf[:]
                # L0: h0(t-1) ready (pre -- fills the PE during the previous
                # step's MLP/pred chain), pred arrives late (emitted last)
                h0, c0 = cell(
                    f"d0_{t}", 0, [(s_wd0h, h0)], [], (s_wd0x, xt),
                    c0, s_lb[2], False,
                    x_mode=("last" if t == 0 else "packed_last"),
                )
                h1, c1 = cell(
                    f"d1_{t}", 1, [(s_wd1h, h1)], [(s_wd1x, h0)], None,
                    c1, s_lb[3], False,
                )

                # MLP head: relu(Wp1 @ h1 + bp1) -> sigmoid(Wp2 @ . + bp2)
                m1_ps = zp.tile([HH, BC], F32, tag="z_f", bufs=1,
                                name=f"m1ps_{t}")
                for k in range(HT):
                    nc.tensor.matmul(
                        m1_ps[:], s_wp1[:, k, :], h1[:, k, :],
                        start=(k == 0), stop=(k == HT - 1),
                    )
                # relu on DVE: (m1 + bp1) max 0 -- keeps ACT off the chain
                m1_sb = gp.tile([HH, BC], BF16, tag="m1sb", name=f"m1sb_{t}")
                nc.vector.scalar_tensor_tensor(
                    m1_sb[:], m1_ps[:], s_bp1[:, 0:1], s_zero[:],
                    ALU.add, ALU.max,
                )
                # m2 as 4 concurrent col-group matmuls: pred lands at
                # partition bases 0/32/64/96 so the next step's packed L0
                # input matmuls can read their row-group replicas directly.
                m2_ps = zp.tile([128, BC], F32, tag="z_i", bufs=1,
                                name=f"m2ps_{t}")
                for g in range(4):
                    nc.tensor.matmul(
                        m2_ps[32 * g : 32 * g + OUT, :], s_wp2[:],
                        m1_sb[:], start=True, stop=True,
                        tile_position=(0, 32 * g),
                        skip_group_check=True,
                    )
                # next-step matmul input first (bf16, all four replicas in
                # one activation -- partitions are free parallelism), then
                # the fp32 DMA copy
                if t < T - 1:
                    pred_bf = gp.tile([128, BC], BF16, tag="predbf",
                                      name=f"predbf_{t}")
                    nc.scalar.activation(pred_bf[:], m2_ps[:], AF.Sigmoid,
                                         bias=s_bp2[:, 0:1])
                pred_f = gp.tile([OUT, BC], F32, tag="predf", name=f"predf_{t}")
                nc.scalar.activation(pred_f[:], m2_ps[0:OUT, :], AF.Sigmoid,
                                     bias=s_bp2[0:OUT, 0:1])
                nc.sync.dma_start(out_r[:, t, :], pred_f[:])

        for _rep in range(repeats):
            emit_forward()

    nc.compile()
    return nc


def _prep_shared(inputs):
    f32 = lambda k: np.asarray(inputs[k], np.float32)
    shared = {
        "we0x": np.ascontiguousarray(f32("enc_Wih0").T).astype(_BF),
        "we0h": _pack_whT(f32("enc_Whh0")),
        "we1x": _pack_whT(f32("enc_Wih1")),
        "we1h": _pack_whT(f32("enc_Whh1")),
        "wd0x": _pack_rep(f32("dec_Wih0")),
        "wd0h": _pack_whT(f32("dec_Whh0")),
        "wd1x": _pack_whT(f32("dec_Wih1")),
        "wd1h": _pack_whT(f32("dec_Whh1")),
        "wp1": _pack_whT(f32("Wp1")),
        "wp2": np.ascontiguousarray(f32("Wp2").T).astype(_BF),
        "bp1": np.ascontiguousarray(f32("bp1").reshape(HH, 1)),
        "bp2": _pack_bias_rep(f32("bp2").reshape(OUT)),
    }
    lstm_biases = [f32("enc_b0"), f32("enc_b1"), f32("dec_b0"), f32("dec_b1")]
    flags = tuple(bool(np.any(b != 0)) for b in lstm_biases)
    for li, (b, flag) in enumerate(zip(lstm_biases, flags)):
        if flag:
            shared[f"lstmbias{li}"] = _pack_bias(b)
    return shared, flags


def _make_in_maps(inputs):
    x = np.asarray(inputs["x"], np.float32)
    assert x.shape == (B, S, IN), x.shape
    shared, _ = _prep_shared(inputs)
    in_maps = []
    for c in range(NCORES):
        xc = x[c * BC : (c + 1) * BC]                       # [BC, S, IN]
        xT = np.ascontiguousarray(xc.transpose(2, 1, 0))    # [IN, S, BC]
        in_maps.append({"xT": xT.reshape(IN, S * BC).astype(_BF), **shared})
    return in_maps


def kernel(**inputs):
    T = int(np.asarray(inputs["target_length"]))

    _, flags = _prep_shared(inputs)
    key = (T, flags)
    if key not in _cache:
        _cache[key] = _build(T, flags)
    nc = _cache[key]

    in_maps = _make_in_maps(inputs)

    res = run_bass_kernel_spmd(nc, in_maps, list(range(NCORES)))
    return np.concatenate(
        [res.results[i]["out"] for i in range(NCORES)], axis=0
    ).astype(np.float32)
